# revision 1
# baseline (speedup 1.0000x reference)
"""Trainium2 Bass kernel for a tanh-RNN (Elman) with output projection.

Reference semantics (fp32):
    W_x = W_ih[:, :1024]; W_h = W_ih[:, 1024:]
    h_t   = tanh(x_t @ W_x.T + h_{t-1} @ W_h.T + b_ih)     # (B, H)
    out_t = h_t @ W_ho.T + b_ho                            # (B, O)
Shapes: x (512, 64, 1024), h0 (64, 1024), W_ih (1024, 2048), b_ih (1024,),
W_ho (512, 1024), b_ho (512,) -> out (512, 64, 512).

Strategy: data-parallel over batch (64 -> 8 per core on 8 NeuronCores),
weights replicated. All heavy layout work is done on the host (free):
x arrives pre-transposed as xT [I, S*B_LOC] bf16, weights pre-transposed;
the output is produced as outT [O, S*B_LOC] and transposed back on host.

Per core, everything is computed in "transposed" layout with the
contraction dim on SBUF partitions:
  phase 1: U' = (2^9 W_x) @ xT + 2^9 b_ih for all timesteps; resident in
           SBUF ([128, S*64] bf16, (t, ho, b) column layout).
  phase 2: 512 serial steps. h^T block [128, 64] (col = ho*8 + b) in a
           SBUF hist tile [128, (S+1)*64]. Per step, NG PSUM banks x
           (ho-groups x 8 hi) weight-stationary matmuls; W_h tiles are
           fp8 e3m4 scaled by 2^9 (fast-weight-load 4x, h moving bf16;
           rel err ~9e-3 end to end). One DVE add of U' + one ScalarE
           tanh(scale=2^-9) per bank group.
  phase 3: outT = W_ho @ h^T from the resident hist (3D moving APs),
           bias via ScalarE, straight DMA out. No on-chip transposes.
Phases 1 and 3 are interleaved into the phase-2 step stream (phase 2 is
LDWEIGHTS-bound, so the N=512 matmuls ride in idle MM-datapath slots):
chunk c's steps carry phase-1 MMs for chunk c+1 and phase-3 MMs for
chunk c-1.
"""
import sys
sys.path.insert(0, "/opt/trn_rl_repo")

import numpy as np
import ml_dtypes

from concourse import bacc
import concourse.mybir as mybir
from concourse.tile import TileContext

N_CORES = 8
S = 512
B = 64
B_LOC = B // N_CORES          # 8
I = 1024
H = 1024
O = 512
KI = I // 128                 # 8 i-tiles
KH = H // 128                 # 8 h-tiles
KO = O // 128                 # 4 o-tiles
R = S * B_LOC                 # 4096 cols (t-major, b-minor)
HB = KH * B_LOC               # 64 columns of the h^T block (ho*8 + b)
WSCALE_POW = 9                # W_h, W_x, b_ih scaled by 2^9 (fp8 range)
NG = 4                        # phase-2 PSUM bank groups (override via ng=)
BF = mybir.dt.bfloat16
F8 = mybir.dt.float8e3
F32 = mybir.dt.float32

TPC = 512 // B_LOC            # timesteps per phase-1/3 chunk (64)


def build_nc(s_run=S, kernel_reps=1, ng=NG):
    nc = bacc.Bacc(None, target_bir_lowering=False, debug=False)
    xT = nc.dram_tensor("xT", [I, R], BF, kind="ExternalInput")
    h0b = nc.dram_tensor("h0b", [128, HB], BF, kind="ExternalInput")
    wxT = nc.dram_tensor("wxT", [I, H], BF, kind="ExternalInput")
    whT8 = nc.dram_tensor("whT8", [H, H], F8, kind="ExternalInput")
    woT = nc.dram_tensor("woT", [H, O], BF, kind="ExternalInput")
    bih2 = nc.dram_tensor("bih2", [H], F32, kind="ExternalInput")
    bho = nc.dram_tensor("bho", [O], F32, kind="ExternalInput")
    outT = nc.dram_tensor("outT", [O, R], F32, kind="ExternalOutput")

    n_chunks = s_run // TPC
    hopg = KH // ng

    with TileContext(nc) as tc:
        with (
            tc.tile_pool(name="weights", bufs=1) as pool_w,
            tc.tile_pool(name="small", bufs=1) as pool_small,
            tc.tile_pool(name="big", bufs=1) as pool_big,
            tc.tile_pool(name="p1x", bufs=3) as pool_x,
            tc.tile_pool(name="p3o", bufs=2) as pool_o,
            tc.tile_pool(name="ps_big", bufs=3, space="PSUM") as ps_big,
            tc.tile_pool(name="ps_p2", bufs=(2 if ng <= 2 else 1),
                         space="PSUM") as ps_p2,
        ):
          for _krep in range(kernel_reps):
            # ---- weight preload ----
            wx, wh, wo = {}, {}, {}
            for k in range(KI):
                wx[k] = pool_w.tile([128, H], BF, tag=f"wx{k}", name=f"wx{k}")
                nc.sync.dma_start(out=wx[k][:], in_=wxT[k * 128:(k + 1) * 128, :])
            for k in range(KH):
                wh[k] = pool_w.tile([128, H], F8, tag=f"wh{k}", name=f"wh{k}")
                nc.sync.dma_start(out=wh[k][:], in_=whT8[k * 128:(k + 1) * 128, :])
            for k in range(KH):
                wo[k] = pool_w.tile([128, O], BF, tag=f"wo{k}", name=f"wo{k}")
                nc.sync.dma_start(out=wo[k][:], in_=woT[k * 128:(k + 1) * 128, :])
            bih_t, bho_t = {}, {}
            for m in range(KH):
                t = pool_small.tile([128, 1], F32, tag=f"bih{m}", name=f"bih{m}")
                nc.sync.dma_start(out=t[:], in_=bih2[m * 128:(m + 1) * 128][:, None])
                bih_t[m] = t
            for m in range(KO):
                t = pool_small.tile([128, 1], F32, tag=f"bho{m}", name=f"bho{m}")
                nc.sync.dma_start(out=t[:], in_=bho[m * 128:(m + 1) * 128][:, None])
                bho_t[m] = t

            u_sb = pool_big.tile([128, s_run, KH, B_LOC], BF, tag="u_sb")
            hist = pool_big.tile([128, s_run + 1, KH, B_LOC], BF, tag="hist")
            nc.sync.dma_start(
                out=hist[:, 0, :, :],
                in_=h0b[:].rearrange("p (k b) -> p k b", k=KH),
            )

            def dma_x_chunk(c):
                xsb = {}
                for k in range(KI):
                    xsb[k] = pool_x.tile([128, 512], BF, tag=f"xsb{k}",
                                         name=f"xsb{k}")
                    nc.sync.dma_start(
                        out=xsb[k][:],
                        in_=xT[k * 128:(k + 1) * 128, c * 512:(c + 1) * 512],
                    )
                return xsb

            p1 = {"ps": None}

            def p1_mm(xsb, c, ho, k):
                if k == 0:
                    p1["ps"] = ps_big.tile([128, 512], F32, tag="pbig",
                                           name="p1z")
                nc.tensor.matmul(
                    p1["ps"][:], wx[k][:, ho * 128:(ho + 1) * 128],
                    xsb[k][:], start=(k == 0), stop=(k == KI - 1),
                )
                if k == KI - 1:
                    nc.scalar.activation(
                        u_sb[:, c * TPC:(c + 1) * TPC, ho, :],
                        p1["ps"][:].rearrange("p (t b) -> p t b", b=B_LOC),
                        mybir.ActivationFunctionType.Identity,
                        bias=bih_t[ho][:],
                    )

            p3 = {"ps": None}

            def p3_mm(c, o, k):
                if k == 0:
                    p3["ps"] = ps_big.tile([128, 512], F32, tag="pbig",
                                           name="p3z")
                nc.tensor.matmul(
                    p3["ps"][:], wo[k][:, o * 128:(o + 1) * 128],
                    hist[:, 1 + c * TPC:1 + (c + 1) * TPC, k, :],
                    start=(k == 0), stop=(k == KH - 1),
                )
                if k == KH - 1:
                    osb = pool_o.tile([128, 512], F32, tag="osb")
                    nc.scalar.activation(
                        osb[:], p3["ps"][:],
                        mybir.ActivationFunctionType.Identity,
                        bias=bho_t[o][:],
                    )
                    nc.sync.dma_start(
                        out=outT[o * 128:(o + 1) * 128, c * 512:(c + 1) * 512],
                        in_=osb[:],
                    )

            def p2_group(t, g):
                pz = ps_p2.tile([128, 512], F32, tag=f"pz{g}", name=f"pz{g}")
                for j in range(hopg):
                    ho = g * hopg + j
                    for hi in range(KH):
                        nc.tensor.matmul(
                            pz[:, j * B_LOC:(j + 1) * B_LOC],
                            wh[hi][:, ho * 128:(ho + 1) * 128],
                            hist[:, t, hi, :],
                            start=(j == 0 and hi == 0),
                            stop=(j == hopg - 1 and hi == KH - 1),
                            skip_group_check=True,
                        )
                gs = hopg * B_LOC
                nc.vector.tensor_add(
                    pz[:, 0:gs], pz[:, 0:gs],
                    u_sb[:, t, g * hopg:(g + 1) * hopg, :],
                )
                nc.scalar.activation(
                    hist[:, t + 1, g * hopg:(g + 1) * hopg, :],
                    pz[:, 0:gs],
                    mybir.ActivationFunctionType.Tanh,
                    scale=float(2.0 ** -WSCALE_POW),
                )

            # ---- prologue: x chunks 0..1, phase-1 chunk 0 dense ----
            xsb_ring = {}
            for c in range(min(2, n_chunks)):
                xsb_ring[c] = dma_x_chunk(c)
            for ho in range(KH):
                for k in range(KI):
                    p1_mm(xsb_ring[0], 0, ho, k)

            # ---- main loop: phase 2 with p1(c+1)/p3(c-1) interleaved ----
            for c in range(n_chunks):
                if c + 2 < n_chunks:
                    xsb_ring[c + 2] = dma_x_chunk(c + 2)
                    xsb_ring.pop(c, None)
                for tl in range(TPC):
                    t = c * TPC + tl
                    p2_group(t, 0)
                    if c + 1 < n_chunks:
                        ho, k = divmod(tl, KI)
                        p1_mm(xsb_ring[c + 1], c + 1, ho, k)
                    for g in range(1, ng):
                        p2_group(t, g)
                    if c >= 1 and tl % 2 == 0:
                        o, k = divmod(tl // 2, KH)
                        p3_mm(c - 1, o, k)
            # ---- epilogue: phase 3 for the last chunk ----
            for o in range(KO):
                for k in range(KH):
                    p3_mm(n_chunks - 1, o, k)

    nc.compile()
    return nc


_NC_CACHE = {}


def _get_nc():
    if "nc" not in _NC_CACHE:
        _NC_CACHE["nc"] = build_nc()
    return _NC_CACHE["nc"]


def make_in_maps(x, h0, W_ih, b_ih, W_ho, b_ho):
    scale = float(2.0 ** WSCALE_POW)
    W_x = W_ih[:, :I]
    W_h = W_ih[:, I:]
    wxT = np.ascontiguousarray((W_x.T * scale)).astype(ml_dtypes.bfloat16)
    whT8 = np.clip(np.ascontiguousarray(W_h.T) * scale, -15.0, 15.0
                   ).astype(ml_dtypes.float8_e3m4)
    woT = np.ascontiguousarray(W_ho.T).astype(ml_dtypes.bfloat16)
    bih2 = (b_ih * scale).astype(np.float32)
    bho = b_ho.astype(np.float32)

    xbf = x.astype(ml_dtypes.bfloat16)
    in_maps = []
    for c in range(N_CORES):
        bsl = slice(c * B_LOC, (c + 1) * B_LOC)
        xTc = np.ascontiguousarray(xbf[:, bsl, :].reshape(R, I).T)
        hc = h0[bsl]                               # (B_LOC, H)
        h0bc = np.ascontiguousarray(
            hc.reshape(B_LOC, KH, 128).transpose(2, 1, 0).reshape(128, HB)
        ).astype(ml_dtypes.bfloat16)
        in_maps.append({
            "xT": xTc,
            "h0b": h0bc,
            "wxT": wxT,
            "whT8": whT8,
            "woT": woT,
            "bih2": bih2,
            "bho": bho,
        })
    return in_maps


def kernel(x, h0, W_ih, b_ih, W_ho, b_ho):
    x = np.ascontiguousarray(np.asarray(x, dtype=np.float32))
    h0 = np.ascontiguousarray(np.asarray(h0, dtype=np.float32))
    W_ih = np.ascontiguousarray(np.asarray(W_ih, dtype=np.float32))
    b_ih = np.ascontiguousarray(np.asarray(b_ih, dtype=np.float32))
    W_ho = np.ascontiguousarray(np.asarray(W_ho, dtype=np.float32))
    b_ho = np.ascontiguousarray(np.asarray(b_ho, dtype=np.float32))

    from concourse.bass_utils import run_bass_kernel_spmd

    nc = _get_nc()
    in_maps = make_in_maps(x, h0, W_ih, b_ih, W_ho, b_ho)
    res = run_bass_kernel_spmd(nc, in_maps, list(range(N_CORES)))
    out = np.empty((S, B, O), np.float32)
    for c in range(N_CORES):
        oT = res.results[c]["outT"]               # (O, R)
        out[:, c * B_LOC:(c + 1) * B_LOC, :] = (
            oT.reshape(O, S, B_LOC).transpose(1, 2, 0)
        )
    return out

